# revision 23
# baseline (speedup 1.0000x reference)
"""MultiHeadNonLocalBlock2d on 8 Trainium2 cores.

Sharding: core = (batch b, n-half): 2048 queries x 4096 keys x 4 heads.

Per-core pipeline (cost-model-shaped):
  proj    fp8 DoubleRow (K=256 as 2 planes) -> psum f32
  q/k     ACT Identity+bias fold -> fp8 plane-0 of qT8/kT8 (plane-1 zeros)
  QK      fp8 DoubleRow, zero plane-1 pads K=32->64; out scoresT [keys, q]
  exp     split ACT (exact Exp -> bf16) / DVE (int16 Schraudolph, bits
          reinterpreted as bf16) -- the two engines are the wall
  AV      exp blocks as PE *weights* (ldweights is free), v [keys,33]
          moving incl. ones col -> psum chains [q, 32d | denom]
  norm    DVE reciprocal + stride-0-broadcast multiply -> yn bf16
  yT      PE transposes (col-banded) + ACT copy -> [hd, q]
  z       single K=128 matmul per c-half (bn inv folded into w_z)
  out     DVE z + residual(x + bn shift) -> DMA
"""

import sys

if '/opt/trn_rl_repo' not in sys.path:
    sys.path.insert(0, '/opt/trn_rl_repo')

from contextlib import ExitStack

import ml_dtypes
import numpy as np

import concourse.bass as bass
import concourse.mybir as mybir
import concourse.tile as tile
from concourse import bacc, bass_utils

F32 = mybir.dt.float32
BF16 = mybir.dt.bfloat16
I16 = mybir.dt.int16
FP8 = mybir.dt.float8e4
BF = ml_dtypes.bfloat16
F8 = ml_dtypes.float8_e4m3

B, C, H, W = 4, 256, 64, 64
INTER, HEADS = 128, 4
D = INTER // HEADS          # 32
N = H * W                   # 4096
EPS = 1e-5
NCORE = 8
NH = N // 2                 # queries per core
CH = 512                    # query chunk
NCH = NH // CH              # 4
MT = N // 128               # 32 key tiles
ALPHA = float(D) ** -0.5

MULT = mybir.AluOpType.mult
ADD = mybir.AluOpType.add
Exp = mybir.ActivationFunctionType.Exp
Ident = mybir.ActivationFunctionType.Identity
DR = mybir.MatmulPerfMode.DoubleRow

LOG2E = 1.4426950408889634
A16 = 128.0 * LOG2E
B16 = 127.0 * 128.0 - 4.0

SPLIT_A = 0.521             # ACT share of exp pair-tiles (ch1-3)
SPLIT_CH0 = 0.47            # ch0 is ACT-heavy (folds + v2 builds)
AVLAG = 6                   # AV trails exp by this many pair-steps
BLAG = 12                   # extra lag for the first pops of a chunk


def build():
    nc = bacc.Bacc("TRN2", target_bir_lowering=False, debug=False)

    x8_d = nc.dram_tensor("x8", [128, 2 * N], FP8, kind="ExternalInput")
    xq_d = nc.dram_tensor("xq", [128, 2 * NH], F32, kind="ExternalInput")
    wp8_d = nc.dram_tensor("wp8", [128, 768], FP8, kind="ExternalInput")
    wz_d = nc.dram_tensor("wz", [128, 256], BF16, kind="ExternalInput")
    idn_d = nc.dram_tensor("idn", [128, 128], BF16, kind="ExternalInput")
    bqk_d = nc.dram_tensor("bqk", [128, 2], F32, kind="ExternalInput")
    out_d = nc.dram_tensor("out", [2, 128, NH], F32, kind="ExternalOutput")

    with tile.TileContext(nc) as tc, ExitStack() as ctx:
        const = ctx.enter_context(tc.tile_pool(name="const", bufs=1))
        sb = ctx.enter_context(tc.tile_pool(name="sb", bufs=1))
        exa = ctx.enter_context(tc.tile_pool(name="exa", bufs=10))
        exd = ctx.enter_context(tc.tile_pool(name="exd", bufs=10))
        ynp = ctx.enter_context(tc.tile_pool(name="ynp", bufs=2))
        ytp = ctx.enter_context(tc.tile_pool(name="ytp", bufs=2))
        rcpp = ctx.enter_context(tc.tile_pool(name="rcpp", bufs=2))
        otp = ctx.enter_context(tc.tile_pool(name="otp", bufs=2))

        # ---- DMA preamble (issue order = need order) ----
        bqk_t = const.tile([128, 2], F32, tag="bqk", name="bqk_t")
        nc.gpsimd.dma_start(bqk_t[:], bqk_d.ap())
        wp8_t = const.tile([128, 768], FP8, tag="wp8", name="wp8_t")
        nc.sync.dma_start(wp8_t[:], wp8_d.ap())
        # x8 is plane-major [2, N]; strided 2-plane DMAs (both planes of a
        # col range in one HWDGE slot) keep projection chunk j fed early
        x8_t = const.tile([128, 2 * N], FP8, tag="x8", name="x8_t")
        x8dv = x8_d.ap().rearrange("p (two n) -> p two n", two=2)
        x8tv = x8_t[:].rearrange("p (two n) -> p two n", two=2)
        for c0, c1 in ((0, 512), (512, 1536), (1536, 2560), (2560, N)):
            nc.sync.dma_start(x8tv[:, :, c0:c1], x8dv[:, :, c0:c1])
        wz_t = const.tile([128, 256], BF16, tag="wz", name="wz_t")
        nc.gpsimd.dma_start(wz_t[:], wz_d.ap())
        idn_t = const.tile([128, 128], BF16, tag="idn", name="idn_t")
        nc.gpsimd.dma_start(idn_t[:], idn_d.ap())
        xq_t = const.tile([128, 2 * NH], F32, tag="xq", name="xq_t")
        for c0 in range(0, 2 * NH, 2048):
            nc.gpsimd.dma_start(xq_t[:, c0:c0 + 2048],
                                xq_d.ap()[:, c0:c0 + 2048])

        x8v = x8_t[:].rearrange("p (two n) -> p two n", two=2)       # [128,2,N]
        wq8 = wp8_t[:, 0:256].rearrange("p (two m) -> p two m", two=2)
        wk8 = wp8_t[:, 256:512].rearrange("p (two m) -> p two m", two=2)
        wg8 = wp8_t[:, 512:768].rearrange("p (two m) -> p two m", two=2)

        # ---- persistent SBUF ----
        qT8 = sb.tile([128, 2 * NH], FP8, tag="qT8", name="qT8")     # (2,NH)
        kT8 = sb.tile([128, 2 * N], FP8, tag="kT8", name="kT8")      # (2,N)
        v2 = sb.tile([128, MT * 132], BF16, tag="v2", name="v2")

        # prefetch the ACT Exp table: a no-dep dummy activation up front so
        # the 1283ns LoadActFuncSet runs during the DMA preamble, not on the
        # first fold's critical path
        dummy = sb.tile([128, 1], F32, tag="dummy", name="dummy")
        nc.scalar.activation(dummy[:], dummy[:], Exp)

        # zero planes / ones col (Pool engine, ordered by tile deps)
        nc.gpsimd.memset(kT8[:, N:N + 512], 0.0)          # covers mt 0..3
        nc.gpsimd.memset(qT8[:, NH:NH + CH], 0.0)         # ch0 plane-1
        v2ones = v2[:].rearrange("p (c w) -> p c w", w=33)[:, :, 32:33]
        nc.gpsimd.memset(v2ones, 1.0)
        nc.gpsimd.memset(kT8[:, N + 512:2 * N], 0.0)
        nc.gpsimd.memset(qT8[:, NH + CH:2 * NH], 0.0)

        with tc.tile_pool(name="sc", bufs=3, space="PSUM") as sc, \
             tc.tile_pool(name="av", bufs=1, space="PSUM") as av:

            # ---- projection emitters ----
            def proj_qk(which, j2):
                # two 512-col chunks into one psum pair, one 1024-wide fold
                wsel, dstt, bcol = ((wq8, qT8, 0) if which == 'q'
                                    else (wk8, kT8, 1))
                ps = sc.tile([128, 1024], F32, tag="s", name="qkps")
                for i in range(2):
                    j = 2 * j2 + i
                    nc.tensor.matmul(ps[:, i * CH:(i + 1) * CH], wsel,
                                     x8v[:, :, j * CH:(j + 1) * CH],
                                     start=True, stop=True, perf_mode=DR)
                with nc.allow_low_precision("qk fold fp8"):
                    nc.scalar.activation(dstt[:, 2 * j2 * CH:2 * (j2 + 1) * CH],
                                         ps[:], Ident,
                                         bias=bqk_t[:, bcol:bcol + 1])

            def proj_v(m4):
                # four key tiles 4*m4.. -> v2 cols, one wide strided build
                ps = sc.tile([128, 1024], F32, tag="s", name="vps")
                for i in range(4):
                    m = 4 * m4 + i
                    nc.tensor.matmul(ps[:, i * 128:i * 128 + 128],
                                     x8v[:, :, m * 128:(m + 1) * 128], wg8,
                                     start=True, stop=True, perf_mode=DR)
                dst = v2[:, m4 * 528:(m4 + 1) * 528] \
                    .rearrange("p (c w) -> p c w", w=33)[:, :, 0:32]
                src = ps[:, 0:512].rearrange("p (c w) -> p c w", w=32)
                with nc.allow_low_precision("v2 bf16"):
                    nc.scalar.copy(dst, src)

            qv = qT8[:].rearrange("p (two n) -> p two n", two=2)
            kv = kT8[:].rearrange("p (two n) -> p two n", two=2)

            # ---- AV + epilogue ----
            st_chains = {}

            def do_av(ch, mt, pr, ex, exbf):
                if mt == 0 and pr == 0:
                    st_chains[ch] = av.tile([128, 1024], F32, tag="av",
                                            name=f"chains{ch}")
                chains = st_chains[ch]
                for i in range(2):
                    h = 2 * pr + i
                    for qt in range(4):
                        c = 4 * h + qt
                        nc.tensor.matmul(
                            chains[:, 64 * c:64 * c + 33],
                            exbf[:, i * CH + qt * 128:i * CH + qt * 128 + 128],
                            v2[:, mt * 132 + h * 33:mt * 132 + h * 33 + 33],
                            start=(mt == 0 and c % 8 == 0), stop=(mt == MT - 1),
                            skip_group_check=True)

            def epilogue(ch):
                chains = st_chains.pop(ch)
                cv = chains[:].rearrange("p (c w) -> p c w", w=64)
                rcp = rcpp.tile([128, 16], F32, tag="r", name="rcp")
                nc.vector.reciprocal(
                    rcp[:].rearrange("p (c u) -> p c u", u=1),
                    cv[:, :, 32:33])
                # one fused normalize (chain-major yn: col 32*(4h+qt)) --
                # chains free right after, so next chunk's AV never stalls
                yn = ynp.tile([128, 512], BF16, tag="yn", name="yn")
                with nc.allow_low_precision("normalize bf16"):
                    nc.vector.tensor_tensor(
                        yn[:].rearrange("p (c w) -> p c w", w=32),
                        cv[:, :, 0:32],
                        rcp[:].rearrange("p (c u) -> p c u", u=1)
                        .broadcast_to([128, 16, 32]),
                        op=MULT)
                # whole epilogue lives in the av arena (rotation: chains ->
                # yt -> z -> next chains); sc keeps all three score slots
                ytt = av.tile([128, 1024], F32, tag="av", name="yt")
                ytv = ytt[:, 0:256].bitcast(BF16)       # [128, 512] bf16
                for qt in range(4):
                    for h in range(4):
                        nc.tensor.matmul(
                            ytv[32 * h:32 * h + 32, 128 * qt:128 * qt + 128],
                            yn[:, 32 * (4 * h + qt):32 * (4 * h + qt) + 32],
                            idn_t[:], is_transpose=True,
                            start=False, stop=True, skip_group_check=True,
                            tile_position=(0, 32 * h))
                yT = ytp.tile([128, 512], BF16, tag="yT", name="yT")
                nc.scalar.copy(yT[:], ytv[:])
                zt = av.tile([128, 1024], F32, tag="av", name="zt")
                for mtz in range(2):
                    nc.tensor.matmul(zt[:, 512 * mtz:512 * mtz + 512],
                                     wz_t[:, 128 * mtz:128 * mtz + 128],
                                     yT[:], start=True, stop=True)
                # fused residual add for both c-halves (strided xq view)
                o = otp.tile([128, 1024], F32, tag="o", name="ot")
                xqv = xq_t[:].rearrange("p (m n) -> p m n", m=2)
                nc.vector.tensor_tensor(
                    o[:].rearrange("p (m n) -> p m n", m=2), zt[:],
                    xqv[:, :, ch * CH:(ch + 1) * CH], op=ADD)
                for mtz in range(2):
                    nc.sync.dma_start(out_d.ap()[mtz, :, ch * CH:(ch + 1) * CH],
                                      o[:, 512 * mtz:512 * mtz + 512])

            # ---- main stream ----
            # startup fast path: QK(0,0) needs kT8[0:128] and qT8[0:512];
            # fold those first, the rest of the k chunk right after
            kps0 = sc.tile([128, 1024], F32, tag="s", name="kps0")
            nc.tensor.matmul(kps0[:, 0:CH], wk8, x8v[:, :, 0:CH],
                             start=True, stop=True, perf_mode=DR)
            qps0 = sc.tile([128, 1024], F32, tag="s", name="qps0")
            nc.tensor.matmul(qps0[:, 0:CH], wq8, x8v[:, :, 0:CH],
                             start=True, stop=True, perf_mode=DR)
            with nc.allow_low_precision("startup folds fp8"):
                nc.scalar.activation(kT8[:, 0:128], kps0[:, 0:128], Ident,
                                     bias=bqk_t[:, 1:2])
                nc.scalar.activation(qT8[:, 0:CH], qps0[:, 0:CH], Ident,
                                     bias=bqk_t[:, 0:1])
                nc.scalar.activation(kT8[:, 128:CH], kps0[:, 128:CH], Ident,
                                     bias=bqk_t[:, 1:2])
            # second half of the first k/q pair chunks
            kps1 = sc.tile([128, 1024], F32, tag="s", name="kps1")
            nc.tensor.matmul(kps1[:, 0:CH], wk8, x8v[:, :, CH:2 * CH],
                             start=True, stop=True, perf_mode=DR)
            nc.tensor.matmul(kps1[:, CH:2 * CH], wq8, x8v[:, :, CH:2 * CH],
                             start=True, stop=True, perf_mode=DR)
            with nc.allow_low_precision("startup folds fp8"):
                nc.scalar.activation(kT8[:, CH:2 * CH], kps1[:, 0:CH], Ident,
                                     bias=bqk_t[:, 1:2])
                nc.scalar.activation(qT8[:, CH:2 * CH], kps1[:, CH:2 * CH],
                                     Ident, bias=bqk_t[:, 0:1])
            proj_v(0)

            queue = []
            acc = [0.0]

            def pick_act(ch, mt, pr):
                step = 2 * mt + pr
                if step >= 62:
                    return True
                acc[0] += SPLIT_CH0 if ch == 0 else SPLIT_A
                if acc[0] >= 1.0:
                    acc[0] -= 1.0
                    return True
                return False

            def pop_av():
                ch, mt, pr, ex, exbf = queue.pop(0)
                do_av(ch, mt, pr, ex, exbf)
                if mt == MT - 1 and pr == 1:
                    epilogue(ch)

            for ch in range(NCH):
                for mt in range(MT):
                    if ch == 0:
                        if mt in (5, 13, 21):
                            proj_qk('k', mt // 8 + 1)
                        if mt == 9:
                            proj_qk('q', 1)
                        if mt in (2, 6, 10, 14, 18, 22, 26):
                            proj_v(mt // 4 + 1)

                    for pr in range(2):
                        s = sc.tile([128, 1024], F32, tag="s", name="sps")
                        for i in range(2):
                            h = 2 * pr + i
                            nc.tensor.matmul(
                                s[:, i * CH:(i + 1) * CH],
                                kv[32 * h:32 * h + 32, :, mt * 128:(mt + 1) * 128],
                                qv[32 * h:32 * h + 32, :, ch * CH:(ch + 1) * CH],
                                start=True, stop=True, perf_mode=DR,
                                tile_position=(32 * h, 0))
                        if pick_act(ch, mt, pr):
                            ex = exa.tile([128, 1024], BF16, tag="e", name="exa")
                            nc.scalar.activation(ex[:], s[:], Exp)
                            exbf = ex[:]
                        else:
                            ex = exd.tile([128, 1024], I16, tag="e", name="exd")
                            with nc.allow_low_precision("schraudolph exp"):
                                nc.vector.tensor_scalar(ex[:], s[:], A16, B16,
                                                        op0=MULT, op1=ADD)
                            exbf = ex[:].bitcast(BF16)
                        queue.append((ch, mt, pr, ex, exbf))
                        # the first pop of a chunk blocks PE on the chains
                        # buffer (freed by the previous epilogue's normalize);
                        # defer it so the QKs emitted before it keep both exp
                        # engines fed across the boundary
                        while queue:
                            hch, hmt, hpr = queue[0][:3]
                            lag = BLAG if (hmt == 0 and hch > 0) else AVLAG
                            cur = (ch * MT + mt) * 2 + pr
                            head = (hch * MT + hmt) * 2 + hpr
                            if cur - head < lag:
                                break
                            pop_av()
            while queue:
                pop_av()

    nc.compile()
    return nc


_NC = None


def _get_nc():
    global _NC
    if _NC is None:
        _NC = build()
    return _NC


def _in_maps(x, w_theta, b_theta, w_phi, b_phi, w_g, b_g, w_z, b_z,
             bn_gamma, bn_beta, bn_mean, bn_var):
    sqa = np.float32(np.sqrt(ALPHA))
    inv = (np.asarray(bn_gamma, np.float32)
           / np.sqrt(np.asarray(bn_var, np.float32) + EPS))
    shift = ((np.asarray(w_z, np.float32) @ np.asarray(b_g, np.float32)
              + np.asarray(b_z, np.float32)) * inv
             + np.asarray(bn_beta, np.float32)
             - np.asarray(bn_mean, np.float32) * inv)

    def pack_w(w):  # [INTER, C] -> [128, 256] lhsT planes, fp8
        wT = np.asarray(w, np.float32).T           # [C, INTER]
        return np.concatenate([wT[:128], wT[128:]], axis=1)

    wq = pack_w(np.asarray(w_theta, np.float32) * sqa)
    wk = pack_w(np.asarray(w_phi, np.float32) * sqa)
    wg = pack_w(np.asarray(w_g, np.float32))
    wp8 = np.ascontiguousarray(
        np.concatenate([wq, wk, wg], axis=1)).astype(F8)
    wzs = np.ascontiguousarray(
        (np.asarray(w_z, np.float32) * inv[:, None]).T.astype(BF))  # [128,256]
    bqk = np.ascontiguousarray(np.stack(
        [np.asarray(b_theta, np.float32) * sqa,
         np.asarray(b_phi, np.float32) * sqa], axis=1))
    idn = np.eye(128, dtype=np.float32).astype(BF)

    xr = np.asarray(x, np.float32).reshape(B, C, N)
    shared = {"wp8": wp8, "wz": wzs, "bqk": bqk, "idn": idn}
    maps = []
    for core in range(NCORE):
        b_, half = divmod(core, 2)
        n0 = half * NH
        n1 = NH - n0
        xre = np.concatenate([xr[b_][:, n0:n0 + NH], xr[b_][:, n1:n1 + NH]],
                             axis=1)            # own query half first
        xs = xre.astype(F8)
        x8 = np.empty((128, 2 * N), F8)
        x8[:, 0:N] = xs[:128]
        x8[:, N:2 * N] = xs[128:]
        xqr = xre[:, 0:NH] + shift[:, None]
        xqp = np.concatenate([xqr[:128], xqr[128:]], axis=1).astype(np.float32)
        m = dict(shared)
        m["x8"] = np.ascontiguousarray(x8)
        m["xq"] = np.ascontiguousarray(xqp)
        maps.append(m)
    return maps


def kernel(**inputs):
    nc = _get_nc()
    maps = _in_maps(**inputs)
    res = bass_utils.run_bass_kernel_spmd(nc, maps, core_ids=list(range(NCORE)))
    out = np.empty((B, C, N), np.float32)
    for core in range(NCORE):
        b_, half = divmod(core, 2)
        n0 = half * NH
        out[b_][:, n0:n0 + NH] = res.results[core]["out"].reshape(C, NH)
    return out.reshape(B, C, H, W)


# revision 24
# speedup vs baseline: 1.0066x; 1.0066x over previous
"""MultiHeadNonLocalBlock2d on 8 Trainium2 cores.

Sharding: core = (batch b, n-half): 2048 queries x 4096 keys x 4 heads.

Per-core pipeline (cost-model-shaped):
  proj    fp8 DoubleRow (K=256 as 2 planes) -> psum f32
  q/k     ACT Identity+bias fold -> fp8 plane-0 of qT8/kT8 (plane-1 zeros)
  QK      fp8 DoubleRow, zero plane-1 pads K=32->64; out scoresT [keys, q]
  exp     split ACT (exact Exp -> bf16) / DVE (int16 Schraudolph, bits
          reinterpreted as bf16) -- the two engines are the wall
  AV      exp blocks as PE *weights* (ldweights is free), v [keys,33]
          moving incl. ones col -> psum chains [q, 32d | denom]
  norm    DVE reciprocal + stride-0-broadcast multiply -> yn bf16
  yT      PE transposes (col-banded) + ACT copy -> [hd, q]
  z       single K=128 matmul per c-half (bn inv folded into w_z)
  out     DVE z + residual(x + bn shift) -> DMA
"""

import sys

if '/opt/trn_rl_repo' not in sys.path:
    sys.path.insert(0, '/opt/trn_rl_repo')

from contextlib import ExitStack

import ml_dtypes
import numpy as np

import concourse.bass as bass
import concourse.mybir as mybir
import concourse.tile as tile
from concourse import bacc, bass_utils

F32 = mybir.dt.float32
BF16 = mybir.dt.bfloat16
I16 = mybir.dt.int16
FP8 = mybir.dt.float8e4
BF = ml_dtypes.bfloat16
F8 = ml_dtypes.float8_e4m3

B, C, H, W = 4, 256, 64, 64
INTER, HEADS = 128, 4
D = INTER // HEADS          # 32
N = H * W                   # 4096
EPS = 1e-5
NCORE = 8
NH = N // 2                 # queries per core
CH = 512                    # query chunk
NCH = NH // CH              # 4
MT = N // 128               # 32 key tiles
ALPHA = float(D) ** -0.5

MULT = mybir.AluOpType.mult
ADD = mybir.AluOpType.add
Exp = mybir.ActivationFunctionType.Exp
Ident = mybir.ActivationFunctionType.Identity
DR = mybir.MatmulPerfMode.DoubleRow

LOG2E = 1.4426950408889634
A16 = 128.0 * LOG2E
B16 = 127.0 * 128.0 - 4.0

SPLIT_A = 0.513             # ACT share of exp pair-tiles (ch1-3)
SPLIT_CH0 = 0.47            # ch0 is ACT-heavy (folds + v2 builds)
AVLAG = 6                   # AV trails exp by this many pair-steps
BLAG = 12                   # extra lag for the first pops of a chunk


def build():
    nc = bacc.Bacc("TRN2", target_bir_lowering=False, debug=False)

    x8_d = nc.dram_tensor("x8", [128, 2 * N], FP8, kind="ExternalInput")
    xq_d = nc.dram_tensor("xq", [128, 2 * NH], F32, kind="ExternalInput")
    wp8_d = nc.dram_tensor("wp8", [128, 768], FP8, kind="ExternalInput")
    wz_d = nc.dram_tensor("wz", [128, 256], BF16, kind="ExternalInput")
    idn_d = nc.dram_tensor("idn", [128, 128], BF16, kind="ExternalInput")
    bqk_d = nc.dram_tensor("bqk", [128, 2], F32, kind="ExternalInput")
    out_d = nc.dram_tensor("out", [2, 128, NH], F32, kind="ExternalOutput")

    with tile.TileContext(nc) as tc, ExitStack() as ctx:
        const = ctx.enter_context(tc.tile_pool(name="const", bufs=1))
        sb = ctx.enter_context(tc.tile_pool(name="sb", bufs=1))
        exa = ctx.enter_context(tc.tile_pool(name="exa", bufs=10))
        exd = ctx.enter_context(tc.tile_pool(name="exd", bufs=10))
        ynp = ctx.enter_context(tc.tile_pool(name="ynp", bufs=2))
        ytp = ctx.enter_context(tc.tile_pool(name="ytp", bufs=2))
        rcpp = ctx.enter_context(tc.tile_pool(name="rcpp", bufs=2))
        otp = ctx.enter_context(tc.tile_pool(name="otp", bufs=2))

        # ---- DMA preamble (issue order = need order) ----
        bqk_t = const.tile([128, 2], F32, tag="bqk", name="bqk_t")
        nc.gpsimd.dma_start(bqk_t[:], bqk_d.ap())
        wp8_t = const.tile([128, 768], FP8, tag="wp8", name="wp8_t")
        nc.sync.dma_start(wp8_t[:], wp8_d.ap())
        # x8 is plane-major [2, N]; strided 2-plane DMAs (both planes of a
        # col range in one HWDGE slot) keep projection chunk j fed early
        x8_t = const.tile([128, 2 * N], FP8, tag="x8", name="x8_t")
        x8dv = x8_d.ap().rearrange("p (two n) -> p two n", two=2)
        x8tv = x8_t[:].rearrange("p (two n) -> p two n", two=2)
        for c0, c1 in ((0, 512), (512, 1536), (1536, 2560), (2560, N)):
            nc.sync.dma_start(x8tv[:, :, c0:c1], x8dv[:, :, c0:c1])
        wz_t = const.tile([128, 256], BF16, tag="wz", name="wz_t")
        nc.gpsimd.dma_start(wz_t[:], wz_d.ap())
        idn_t = const.tile([128, 128], BF16, tag="idn", name="idn_t")
        nc.gpsimd.dma_start(idn_t[:], idn_d.ap())
        xq_t = const.tile([128, 2 * NH], F32, tag="xq", name="xq_t")
        for c0 in range(0, 2 * NH, 2048):
            nc.gpsimd.dma_start(xq_t[:, c0:c0 + 2048],
                                xq_d.ap()[:, c0:c0 + 2048])

        x8v = x8_t[:].rearrange("p (two n) -> p two n", two=2)       # [128,2,N]
        wq8 = wp8_t[:, 0:256].rearrange("p (two m) -> p two m", two=2)
        wk8 = wp8_t[:, 256:512].rearrange("p (two m) -> p two m", two=2)
        wg8 = wp8_t[:, 512:768].rearrange("p (two m) -> p two m", two=2)

        # ---- persistent SBUF ----
        qT8 = sb.tile([128, 2 * NH], FP8, tag="qT8", name="qT8")     # (2,NH)
        kT8 = sb.tile([128, 2 * N], FP8, tag="kT8", name="kT8")      # (2,N)
        v2 = sb.tile([128, MT * 132], BF16, tag="v2", name="v2")

        # prefetch the ACT Exp table: a no-dep dummy activation up front so
        # the 1283ns LoadActFuncSet runs during the DMA preamble, not on the
        # first fold's critical path
        dummy = sb.tile([128, 1], F32, tag="dummy", name="dummy")
        nc.scalar.activation(dummy[:], dummy[:], Exp)

        # zero planes / ones col (Pool engine, ordered by tile deps)
        nc.gpsimd.memset(kT8[:, N:N + 512], 0.0)          # covers mt 0..3
        nc.gpsimd.memset(qT8[:, NH:NH + CH], 0.0)         # ch0 plane-1
        v2ones = v2[:].rearrange("p (c w) -> p c w", w=33)[:, :, 32:33]
        nc.gpsimd.memset(v2ones, 1.0)
        nc.gpsimd.memset(kT8[:, N + 512:2 * N], 0.0)
        nc.gpsimd.memset(qT8[:, NH + CH:2 * NH], 0.0)

        with tc.tile_pool(name="sc", bufs=3, space="PSUM") as sc, \
             tc.tile_pool(name="av", bufs=1, space="PSUM") as av:

            # ---- projection emitters ----
            def proj_qk(which, j2):
                # two 512-col chunks into one psum pair, one 1024-wide fold
                wsel, dstt, bcol = ((wq8, qT8, 0) if which == 'q'
                                    else (wk8, kT8, 1))
                ps = sc.tile([128, 1024], F32, tag="s", name="qkps")
                for i in range(2):
                    j = 2 * j2 + i
                    nc.tensor.matmul(ps[:, i * CH:(i + 1) * CH], wsel,
                                     x8v[:, :, j * CH:(j + 1) * CH],
                                     start=True, stop=True, perf_mode=DR)
                with nc.allow_low_precision("qk fold fp8"):
                    nc.scalar.activation(dstt[:, 2 * j2 * CH:2 * (j2 + 1) * CH],
                                         ps[:], Ident,
                                         bias=bqk_t[:, bcol:bcol + 1])

            def proj_v(m4):
                # four key tiles 4*m4.. -> v2 cols, one wide strided build
                ps = sc.tile([128, 1024], F32, tag="s", name="vps")
                for i in range(4):
                    m = 4 * m4 + i
                    nc.tensor.matmul(ps[:, i * 128:i * 128 + 128],
                                     x8v[:, :, m * 128:(m + 1) * 128], wg8,
                                     start=True, stop=True, perf_mode=DR)
                dst = v2[:, m4 * 528:(m4 + 1) * 528] \
                    .rearrange("p (c w) -> p c w", w=33)[:, :, 0:32]
                src = ps[:, 0:512].rearrange("p (c w) -> p c w", w=32)
                with nc.allow_low_precision("v2 bf16"):
                    nc.scalar.copy(dst, src)

            qv = qT8[:].rearrange("p (two n) -> p two n", two=2)
            kv = kT8[:].rearrange("p (two n) -> p two n", two=2)

            # ---- AV + epilogue ----
            st_chains = {}

            def do_av(ch, mt, pr, ex, exbf):
                if mt == 0 and pr == 0:
                    st_chains[ch] = av.tile([128, 1024], F32, tag="av",
                                            name=f"chains{ch}")
                chains = st_chains[ch]
                for i in range(2):
                    h = 2 * pr + i
                    for qt in range(4):
                        c = 4 * h + qt
                        nc.tensor.matmul(
                            chains[:, 64 * c:64 * c + 33],
                            exbf[:, i * CH + qt * 128:i * CH + qt * 128 + 128],
                            v2[:, mt * 132 + h * 33:mt * 132 + h * 33 + 33],
                            start=(mt == 0 and c % 8 == 0), stop=(mt == MT - 1),
                            skip_group_check=True)

            def epilogue(ch):
                chains = st_chains.pop(ch)
                cv = chains[:].rearrange("p (c w) -> p c w", w=64)
                rcp = rcpp.tile([128, 16], F32, tag="r", name="rcp")
                nc.vector.reciprocal(
                    rcp[:].rearrange("p (c u) -> p c u", u=1),
                    cv[:, :, 32:33])
                # one fused normalize (chain-major yn: col 32*(4h+qt)) --
                # chains free right after, so next chunk's AV never stalls
                yn = ynp.tile([128, 512], BF16, tag="yn", name="yn")
                with nc.allow_low_precision("normalize bf16"):
                    nc.vector.tensor_tensor(
                        yn[:].rearrange("p (c w) -> p c w", w=32),
                        cv[:, :, 0:32],
                        rcp[:].rearrange("p (c u) -> p c u", u=1)
                        .broadcast_to([128, 16, 32]),
                        op=MULT)
                # whole epilogue lives in the av arena (rotation: chains ->
                # yt -> z -> next chains); sc keeps all three score slots
                ytt = av.tile([128, 1024], F32, tag="av", name="yt")
                ytv = ytt[:, 0:256].bitcast(BF16)       # [128, 512] bf16
                for qt in range(4):
                    for h in range(4):
                        nc.tensor.matmul(
                            ytv[32 * h:32 * h + 32, 128 * qt:128 * qt + 128],
                            yn[:, 32 * (4 * h + qt):32 * (4 * h + qt) + 32],
                            idn_t[:], is_transpose=True,
                            start=False, stop=True, skip_group_check=True,
                            tile_position=(0, 32 * h))
                yT = ytp.tile([128, 512], BF16, tag="yT", name="yT")
                nc.scalar.copy(yT[:], ytv[:])
                zt = av.tile([128, 1024], F32, tag="av", name="zt")
                for mtz in range(2):
                    nc.tensor.matmul(zt[:, 512 * mtz:512 * mtz + 512],
                                     wz_t[:, 128 * mtz:128 * mtz + 128],
                                     yT[:], start=True, stop=True)
                # fused residual add for both c-halves (strided xq view)
                o = otp.tile([128, 1024], F32, tag="o", name="ot")
                xqv = xq_t[:].rearrange("p (m n) -> p m n", m=2)
                nc.vector.tensor_tensor(
                    o[:].rearrange("p (m n) -> p m n", m=2), zt[:],
                    xqv[:, :, ch * CH:(ch + 1) * CH], op=ADD)
                for mtz in range(2):
                    nc.sync.dma_start(out_d.ap()[mtz, :, ch * CH:(ch + 1) * CH],
                                      o[:, 512 * mtz:512 * mtz + 512])

            # ---- main stream ----
            # startup fast path: QK(0,0) needs kT8[0:128] and qT8[0:512];
            # fold those first, the rest of the k chunk right after
            kps0 = sc.tile([128, 1024], F32, tag="s", name="kps0")
            nc.tensor.matmul(kps0[:, 0:CH], wk8, x8v[:, :, 0:CH],
                             start=True, stop=True, perf_mode=DR)
            qps0 = sc.tile([128, 1024], F32, tag="s", name="qps0")
            nc.tensor.matmul(qps0[:, 0:CH], wq8, x8v[:, :, 0:CH],
                             start=True, stop=True, perf_mode=DR)
            with nc.allow_low_precision("startup folds fp8"):
                nc.scalar.activation(kT8[:, 0:128], kps0[:, 0:128], Ident,
                                     bias=bqk_t[:, 1:2])
                nc.scalar.activation(qT8[:, 0:CH], qps0[:, 0:CH], Ident,
                                     bias=bqk_t[:, 0:1])
                nc.scalar.activation(kT8[:, 128:CH], kps0[:, 128:CH], Ident,
                                     bias=bqk_t[:, 1:2])
            # second half of the first k/q pair chunks
            kps1 = sc.tile([128, 1024], F32, tag="s", name="kps1")
            nc.tensor.matmul(kps1[:, 0:CH], wk8, x8v[:, :, CH:2 * CH],
                             start=True, stop=True, perf_mode=DR)
            nc.tensor.matmul(kps1[:, CH:2 * CH], wq8, x8v[:, :, CH:2 * CH],
                             start=True, stop=True, perf_mode=DR)
            with nc.allow_low_precision("startup folds fp8"):
                nc.scalar.activation(kT8[:, CH:2 * CH], kps1[:, 0:CH], Ident,
                                     bias=bqk_t[:, 1:2])
                nc.scalar.activation(qT8[:, CH:2 * CH], kps1[:, CH:2 * CH],
                                     Ident, bias=bqk_t[:, 0:1])
            proj_v(0)

            queue = []
            acc = [0.0]

            def pick_act(ch, mt, pr):
                step = 2 * mt + pr
                if step >= 62:
                    return True
                acc[0] += SPLIT_CH0 if ch == 0 else SPLIT_A
                if acc[0] >= 1.0:
                    acc[0] -= 1.0
                    return True
                return False

            def pop_av():
                ch, mt, pr, ex, exbf = queue.pop(0)
                do_av(ch, mt, pr, ex, exbf)
                if mt == MT - 1 and pr == 1:
                    epilogue(ch)

            for ch in range(NCH):
                for mt in range(MT):
                    if ch == 0:
                        if mt in (5, 13, 21):
                            proj_qk('k', mt // 8 + 1)
                        if mt == 9:
                            proj_qk('q', 1)
                        if mt in (2, 6, 10, 14, 18, 22, 26):
                            proj_v(mt // 4 + 1)

                    for pr in range(2):
                        s = sc.tile([128, 1024], F32, tag="s", name="sps")
                        for i in range(2):
                            h = 2 * pr + i
                            nc.tensor.matmul(
                                s[:, i * CH:(i + 1) * CH],
                                kv[32 * h:32 * h + 32, :, mt * 128:(mt + 1) * 128],
                                qv[32 * h:32 * h + 32, :, ch * CH:(ch + 1) * CH],
                                start=True, stop=True, perf_mode=DR,
                                tile_position=(32 * h, 0))
                        if pick_act(ch, mt, pr):
                            ex = exa.tile([128, 1024], BF16, tag="e", name="exa")
                            nc.scalar.activation(ex[:], s[:], Exp)
                            exbf = ex[:]
                        else:
                            ex = exd.tile([128, 1024], I16, tag="e", name="exd")
                            with nc.allow_low_precision("schraudolph exp"):
                                nc.vector.tensor_scalar(ex[:], s[:], A16, B16,
                                                        op0=MULT, op1=ADD)
                            exbf = ex[:].bitcast(BF16)
                        queue.append((ch, mt, pr, ex, exbf))
                        # the first pop of a chunk blocks PE on the chains
                        # buffer (freed by the previous epilogue's normalize);
                        # defer it so the QKs emitted before it keep both exp
                        # engines fed across the boundary
                        while queue:
                            hch, hmt, hpr = queue[0][:3]
                            lag = BLAG if (hmt == 0 and hch > 0) else AVLAG
                            cur = (ch * MT + mt) * 2 + pr
                            head = (hch * MT + hmt) * 2 + hpr
                            if cur - head < lag:
                                break
                            pop_av()
            while queue:
                pop_av()

    nc.compile()
    return nc


_NC = None


def _get_nc():
    global _NC
    if _NC is None:
        _NC = build()
    return _NC


def _in_maps(x, w_theta, b_theta, w_phi, b_phi, w_g, b_g, w_z, b_z,
             bn_gamma, bn_beta, bn_mean, bn_var):
    sqa = np.float32(np.sqrt(ALPHA))
    inv = (np.asarray(bn_gamma, np.float32)
           / np.sqrt(np.asarray(bn_var, np.float32) + EPS))
    shift = ((np.asarray(w_z, np.float32) @ np.asarray(b_g, np.float32)
              + np.asarray(b_z, np.float32)) * inv
             + np.asarray(bn_beta, np.float32)
             - np.asarray(bn_mean, np.float32) * inv)

    def pack_w(w):  # [INTER, C] -> [128, 256] lhsT planes, fp8
        wT = np.asarray(w, np.float32).T           # [C, INTER]
        return np.concatenate([wT[:128], wT[128:]], axis=1)

    wq = pack_w(np.asarray(w_theta, np.float32) * sqa)
    wk = pack_w(np.asarray(w_phi, np.float32) * sqa)
    wg = pack_w(np.asarray(w_g, np.float32))
    wp8 = np.ascontiguousarray(
        np.concatenate([wq, wk, wg], axis=1)).astype(F8)
    wzs = np.ascontiguousarray(
        (np.asarray(w_z, np.float32) * inv[:, None]).T.astype(BF))  # [128,256]
    bqk = np.ascontiguousarray(np.stack(
        [np.asarray(b_theta, np.float32) * sqa,
         np.asarray(b_phi, np.float32) * sqa], axis=1))
    idn = np.eye(128, dtype=np.float32).astype(BF)

    xr = np.asarray(x, np.float32).reshape(B, C, N)
    shared = {"wp8": wp8, "wz": wzs, "bqk": bqk, "idn": idn}
    maps = []
    for core in range(NCORE):
        b_, half = divmod(core, 2)
        n0 = half * NH
        n1 = NH - n0
        xre = np.concatenate([xr[b_][:, n0:n0 + NH], xr[b_][:, n1:n1 + NH]],
                             axis=1)            # own query half first
        xs = xre.astype(F8)
        x8 = np.empty((128, 2 * N), F8)
        x8[:, 0:N] = xs[:128]
        x8[:, N:2 * N] = xs[128:]
        xqr = xre[:, 0:NH] + shift[:, None]
        xqp = np.concatenate([xqr[:128], xqr[128:]], axis=1).astype(np.float32)
        m = dict(shared)
        m["x8"] = np.ascontiguousarray(x8)
        m["xq"] = np.ascontiguousarray(xqp)
        maps.append(m)
    return maps


def kernel(**inputs):
    nc = _get_nc()
    maps = _in_maps(**inputs)
    res = bass_utils.run_bass_kernel_spmd(nc, maps, core_ids=list(range(NCORE)))
    out = np.empty((B, C, N), np.float32)
    for core in range(NCORE):
        b_, half = divmod(core, 2)
        n0 = half * NH
        out[b_][:, n0:n0 + NH] = res.results[core]["out"].reshape(C, NH)
    return out.reshape(B, C, H, W)


# revision 25
# speedup vs baseline: 1.0089x; 1.0023x over previous
"""MultiHeadNonLocalBlock2d on 8 Trainium2 cores.

Sharding: core = (batch b, n-half): 2048 queries x 4096 keys x 4 heads.

Per-core pipeline (cost-model-shaped):
  proj    fp8 DoubleRow (K=256 as 2 planes) -> psum f32
  q/k     ACT Identity+bias fold -> fp8 plane-0 of qT8/kT8 (plane-1 zeros)
  QK      fp8 DoubleRow, zero plane-1 pads K=32->64; out scoresT [keys, q]
  exp     split ACT (exact Exp -> bf16) / DVE (int16 Schraudolph, bits
          reinterpreted as bf16) -- the two engines are the wall
  AV      exp blocks as PE *weights* (ldweights is free), v [keys,33]
          moving incl. ones col -> psum chains [q, 32d | denom]
  norm    DVE reciprocal + stride-0-broadcast multiply -> yn bf16
  yT      PE transposes (col-banded) + ACT copy -> [hd, q]
  z       single K=128 matmul per c-half (bn inv folded into w_z)
  out     DVE z + residual(x + bn shift) -> DMA
"""

import sys

if '/opt/trn_rl_repo' not in sys.path:
    sys.path.insert(0, '/opt/trn_rl_repo')

from contextlib import ExitStack

import ml_dtypes
import numpy as np

import concourse.bass as bass
import concourse.mybir as mybir
import concourse.tile as tile
from concourse import bacc, bass_utils

F32 = mybir.dt.float32
BF16 = mybir.dt.bfloat16
I16 = mybir.dt.int16
FP8 = mybir.dt.float8e4
BF = ml_dtypes.bfloat16
F8 = ml_dtypes.float8_e4m3

B, C, H, W = 4, 256, 64, 64
INTER, HEADS = 128, 4
D = INTER // HEADS          # 32
N = H * W                   # 4096
EPS = 1e-5
NCORE = 8
NH = N // 2                 # queries per core
CH = 512                    # query chunk
NCH = NH // CH              # 4
MT = N // 128               # 32 key tiles
ALPHA = float(D) ** -0.5

MULT = mybir.AluOpType.mult
ADD = mybir.AluOpType.add
Exp = mybir.ActivationFunctionType.Exp
Ident = mybir.ActivationFunctionType.Identity
DR = mybir.MatmulPerfMode.DoubleRow

LOG2E = 1.4426950408889634
A16 = 128.0 * LOG2E
B16 = 127.0 * 128.0 - 4.0

SPLIT_A = 0.513             # ACT share of exp pair-tiles (ch1-3)
SPLIT_CH0 = 0.50            # ch0 is ACT-heavy (folds + v2 builds)
AVLAG = 6                   # AV trails exp by this many pair-steps
BLAG = 10                   # extra lag for the first pops of a chunk


def build():
    nc = bacc.Bacc("TRN2", target_bir_lowering=False, debug=False)

    x8_d = nc.dram_tensor("x8", [128, 2 * N], FP8, kind="ExternalInput")
    xq_d = nc.dram_tensor("xq", [128, 2 * NH], F32, kind="ExternalInput")
    wp8_d = nc.dram_tensor("wp8", [128, 768], FP8, kind="ExternalInput")
    wz_d = nc.dram_tensor("wz", [128, 256], BF16, kind="ExternalInput")
    idn_d = nc.dram_tensor("idn", [128, 128], BF16, kind="ExternalInput")
    bqk_d = nc.dram_tensor("bqk", [128, 2], F32, kind="ExternalInput")
    out_d = nc.dram_tensor("out", [2, 128, NH], F32, kind="ExternalOutput")

    with tile.TileContext(nc) as tc, ExitStack() as ctx:
        const = ctx.enter_context(tc.tile_pool(name="const", bufs=1))
        sb = ctx.enter_context(tc.tile_pool(name="sb", bufs=1))
        exa = ctx.enter_context(tc.tile_pool(name="exa", bufs=10))
        exd = ctx.enter_context(tc.tile_pool(name="exd", bufs=10))
        ynp = ctx.enter_context(tc.tile_pool(name="ynp", bufs=2))
        ytp = ctx.enter_context(tc.tile_pool(name="ytp", bufs=2))
        rcpp = ctx.enter_context(tc.tile_pool(name="rcpp", bufs=2))
        otp = ctx.enter_context(tc.tile_pool(name="otp", bufs=2))

        # ---- DMA preamble (issue order = need order) ----
        bqk_t = const.tile([128, 2], F32, tag="bqk", name="bqk_t")
        nc.gpsimd.dma_start(bqk_t[:], bqk_d.ap())
        wp8_t = const.tile([128, 768], FP8, tag="wp8", name="wp8_t")
        nc.sync.dma_start(wp8_t[:], wp8_d.ap())
        # x8 is plane-major [2, N]; strided 2-plane DMAs (both planes of a
        # col range in one HWDGE slot) keep projection chunk j fed early
        x8_t = const.tile([128, 2 * N], FP8, tag="x8", name="x8_t")
        x8dv = x8_d.ap().rearrange("p (two n) -> p two n", two=2)
        x8tv = x8_t[:].rearrange("p (two n) -> p two n", two=2)
        for c0, c1 in ((0, 512), (512, 1536), (1536, 2560), (2560, N)):
            nc.sync.dma_start(x8tv[:, :, c0:c1], x8dv[:, :, c0:c1])
        wz_t = const.tile([128, 256], BF16, tag="wz", name="wz_t")
        nc.gpsimd.dma_start(wz_t[:], wz_d.ap())
        idn_t = const.tile([128, 128], BF16, tag="idn", name="idn_t")
        nc.gpsimd.dma_start(idn_t[:], idn_d.ap())
        xq_t = const.tile([128, 2 * NH], F32, tag="xq", name="xq_t")
        for c0 in range(0, 2 * NH, 2048):
            nc.gpsimd.dma_start(xq_t[:, c0:c0 + 2048],
                                xq_d.ap()[:, c0:c0 + 2048])

        x8v = x8_t[:].rearrange("p (two n) -> p two n", two=2)       # [128,2,N]
        wq8 = wp8_t[:, 0:256].rearrange("p (two m) -> p two m", two=2)
        wk8 = wp8_t[:, 256:512].rearrange("p (two m) -> p two m", two=2)
        wg8 = wp8_t[:, 512:768].rearrange("p (two m) -> p two m", two=2)

        # ---- persistent SBUF ----
        qT8 = sb.tile([128, 2 * NH], FP8, tag="qT8", name="qT8")     # (2,NH)
        kT8 = sb.tile([128, 2 * N], FP8, tag="kT8", name="kT8")      # (2,N)
        v2 = sb.tile([128, MT * 132], BF16, tag="v2", name="v2")

        # prefetch the ACT Exp table: a no-dep dummy activation up front so
        # the 1283ns LoadActFuncSet runs during the DMA preamble, not on the
        # first fold's critical path
        dummy = sb.tile([128, 1], F32, tag="dummy", name="dummy")
        nc.scalar.activation(dummy[:], dummy[:], Exp)

        # zero planes / ones col (Pool engine, ordered by tile deps)
        nc.gpsimd.memset(kT8[:, N:N + 512], 0.0)          # covers mt 0..3
        nc.gpsimd.memset(qT8[:, NH:NH + CH], 0.0)         # ch0 plane-1
        v2ones = v2[:].rearrange("p (c w) -> p c w", w=33)[:, :, 32:33]
        nc.gpsimd.memset(v2ones, 1.0)
        nc.gpsimd.memset(kT8[:, N + 512:2 * N], 0.0)
        nc.gpsimd.memset(qT8[:, NH + CH:2 * NH], 0.0)

        with tc.tile_pool(name="sc", bufs=3, space="PSUM") as sc, \
             tc.tile_pool(name="av", bufs=1, space="PSUM") as av:

            # ---- projection emitters ----
            def proj_qk(which, j2):
                # two 512-col chunks into one psum pair, one 1024-wide fold
                wsel, dstt, bcol = ((wq8, qT8, 0) if which == 'q'
                                    else (wk8, kT8, 1))
                ps = sc.tile([128, 1024], F32, tag="s", name="qkps")
                for i in range(2):
                    j = 2 * j2 + i
                    nc.tensor.matmul(ps[:, i * CH:(i + 1) * CH], wsel,
                                     x8v[:, :, j * CH:(j + 1) * CH],
                                     start=True, stop=True, perf_mode=DR)
                with nc.allow_low_precision("qk fold fp8"):
                    nc.scalar.activation(dstt[:, 2 * j2 * CH:2 * (j2 + 1) * CH],
                                         ps[:], Ident,
                                         bias=bqk_t[:, bcol:bcol + 1])

            def proj_v(m4):
                # four key tiles 4*m4.. -> v2 cols, one wide strided build
                ps = sc.tile([128, 1024], F32, tag="s", name="vps")
                for i in range(4):
                    m = 4 * m4 + i
                    nc.tensor.matmul(ps[:, i * 128:i * 128 + 128],
                                     x8v[:, :, m * 128:(m + 1) * 128], wg8,
                                     start=True, stop=True, perf_mode=DR)
                dst = v2[:, m4 * 528:(m4 + 1) * 528] \
                    .rearrange("p (c w) -> p c w", w=33)[:, :, 0:32]
                src = ps[:, 0:512].rearrange("p (c w) -> p c w", w=32)
                with nc.allow_low_precision("v2 bf16"):
                    nc.scalar.copy(dst, src)

            qv = qT8[:].rearrange("p (two n) -> p two n", two=2)
            kv = kT8[:].rearrange("p (two n) -> p two n", two=2)

            # ---- AV + epilogue ----
            st_chains = {}

            def do_av(ch, mt, pr, ex, exbf):
                if mt == 0 and pr == 0:
                    st_chains[ch] = av.tile([128, 1024], F32, tag="av",
                                            name=f"chains{ch}")
                chains = st_chains[ch]
                for i in range(2):
                    h = 2 * pr + i
                    for qt in range(4):
                        c = 4 * h + qt
                        nc.tensor.matmul(
                            chains[:, 64 * c:64 * c + 33],
                            exbf[:, i * CH + qt * 128:i * CH + qt * 128 + 128],
                            v2[:, mt * 132 + h * 33:mt * 132 + h * 33 + 33],
                            start=(mt == 0 and c % 8 == 0), stop=(mt == MT - 1),
                            skip_group_check=True)

            def epilogue(ch):
                chains = st_chains.pop(ch)
                cv = chains[:].rearrange("p (c w) -> p c w", w=64)
                rcp = rcpp.tile([128, 16], F32, tag="r", name="rcp")
                nc.vector.reciprocal(
                    rcp[:].rearrange("p (c u) -> p c u", u=1),
                    cv[:, :, 32:33])
                # one fused normalize (chain-major yn: col 32*(4h+qt)) --
                # chains free right after, so next chunk's AV never stalls
                yn = ynp.tile([128, 512], BF16, tag="yn", name="yn")
                with nc.allow_low_precision("normalize bf16"):
                    nc.vector.tensor_tensor(
                        yn[:].rearrange("p (c w) -> p c w", w=32),
                        cv[:, :, 0:32],
                        rcp[:].rearrange("p (c u) -> p c u", u=1)
                        .broadcast_to([128, 16, 32]),
                        op=MULT)
                # whole epilogue lives in the av arena (rotation: chains ->
                # yt -> z -> next chains); sc keeps all three score slots
                ytt = av.tile([128, 1024], F32, tag="av", name="yt")
                ytv = ytt[:, 0:256].bitcast(BF16)       # [128, 512] bf16
                for qt in range(4):
                    for h in range(4):
                        nc.tensor.matmul(
                            ytv[32 * h:32 * h + 32, 128 * qt:128 * qt + 128],
                            yn[:, 32 * (4 * h + qt):32 * (4 * h + qt) + 32],
                            idn_t[:], is_transpose=True,
                            start=False, stop=True, skip_group_check=True,
                            tile_position=(0, 32 * h))
                yT = ytp.tile([128, 512], BF16, tag="yT", name="yT")
                nc.scalar.copy(yT[:], ytv[:])
                zt = av.tile([128, 1024], F32, tag="av", name="zt")
                for mtz in range(2):
                    nc.tensor.matmul(zt[:, 512 * mtz:512 * mtz + 512],
                                     wz_t[:, 128 * mtz:128 * mtz + 128],
                                     yT[:], start=True, stop=True)
                # fused residual add for both c-halves (strided xq view)
                o = otp.tile([128, 1024], F32, tag="o", name="ot")
                xqv = xq_t[:].rearrange("p (m n) -> p m n", m=2)
                nc.vector.tensor_tensor(
                    o[:].rearrange("p (m n) -> p m n", m=2), zt[:],
                    xqv[:, :, ch * CH:(ch + 1) * CH], op=ADD)
                for mtz in range(2):
                    nc.sync.dma_start(out_d.ap()[mtz, :, ch * CH:(ch + 1) * CH],
                                      o[:, 512 * mtz:512 * mtz + 512])

            # ---- main stream ----
            # startup fast path: QK(0,0) needs kT8[0:128] and qT8[0:512];
            # fold those first, the rest of the k chunk right after
            kps0 = sc.tile([128, 1024], F32, tag="s", name="kps0")
            nc.tensor.matmul(kps0[:, 0:CH], wk8, x8v[:, :, 0:CH],
                             start=True, stop=True, perf_mode=DR)
            qps0 = sc.tile([128, 1024], F32, tag="s", name="qps0")
            nc.tensor.matmul(qps0[:, 0:CH], wq8, x8v[:, :, 0:CH],
                             start=True, stop=True, perf_mode=DR)
            with nc.allow_low_precision("startup folds fp8"):
                nc.scalar.activation(kT8[:, 0:128], kps0[:, 0:128], Ident,
                                     bias=bqk_t[:, 1:2])
                nc.scalar.activation(qT8[:, 0:CH], qps0[:, 0:CH], Ident,
                                     bias=bqk_t[:, 0:1])
                nc.vector.tensor_scalar(kT8[:, 128:CH], kps0[:, 128:CH],
                                        bqk_t[:, 1:2], None, op0=ADD)
            # second half of the first k/q pair chunks
            kps1 = sc.tile([128, 1024], F32, tag="s", name="kps1")
            nc.tensor.matmul(kps1[:, 0:CH], wk8, x8v[:, :, CH:2 * CH],
                             start=True, stop=True, perf_mode=DR)
            nc.tensor.matmul(kps1[:, CH:2 * CH], wq8, x8v[:, :, CH:2 * CH],
                             start=True, stop=True, perf_mode=DR)
            with nc.allow_low_precision("startup folds fp8"):
                nc.vector.tensor_scalar(kT8[:, CH:2 * CH], kps1[:, 0:CH],
                                        bqk_t[:, 1:2], None, op0=ADD)
                nc.vector.tensor_scalar(qT8[:, CH:2 * CH], kps1[:, CH:2 * CH],
                                        bqk_t[:, 0:1], None, op0=ADD)
            proj_v(0)

            queue = []
            acc = [0.0]

            def pick_act(ch, mt, pr):
                step = 2 * mt + pr
                if step >= 62:
                    return True
                acc[0] += SPLIT_CH0 if ch == 0 else SPLIT_A
                if acc[0] >= 1.0:
                    acc[0] -= 1.0
                    return True
                return False

            def pop_av():
                ch, mt, pr, ex, exbf = queue.pop(0)
                do_av(ch, mt, pr, ex, exbf)
                if mt == MT - 1 and pr == 1:
                    epilogue(ch)

            for ch in range(NCH):
                for mt in range(MT):
                    if ch == 0:
                        if mt in (5, 13, 21):
                            proj_qk('k', mt // 8 + 1)
                        if mt == 9:
                            proj_qk('q', 1)
                        if mt in (2, 6, 10, 14, 18, 22, 26):
                            proj_v(mt // 4 + 1)

                    for pr in range(2):
                        s = sc.tile([128, 1024], F32, tag="s", name="sps")
                        for i in range(2):
                            h = 2 * pr + i
                            nc.tensor.matmul(
                                s[:, i * CH:(i + 1) * CH],
                                kv[32 * h:32 * h + 32, :, mt * 128:(mt + 1) * 128],
                                qv[32 * h:32 * h + 32, :, ch * CH:(ch + 1) * CH],
                                start=True, stop=True, perf_mode=DR,
                                tile_position=(32 * h, 0))
                        if pick_act(ch, mt, pr):
                            ex = exa.tile([128, 1024], BF16, tag="e", name="exa")
                            nc.scalar.activation(ex[:], s[:], Exp)
                            exbf = ex[:]
                        else:
                            ex = exd.tile([128, 1024], I16, tag="e", name="exd")
                            with nc.allow_low_precision("schraudolph exp"):
                                nc.vector.tensor_scalar(ex[:], s[:], A16, B16,
                                                        op0=MULT, op1=ADD)
                            exbf = ex[:].bitcast(BF16)
                        queue.append((ch, mt, pr, ex, exbf))
                        # the first pop of a chunk blocks PE on the chains
                        # buffer (freed by the previous epilogue's normalize);
                        # defer it so the QKs emitted before it keep both exp
                        # engines fed across the boundary
                        while queue:
                            hch, hmt, hpr = queue[0][:3]
                            lag = BLAG if (hmt == 0 and hch > 0) else AVLAG
                            cur = (ch * MT + mt) * 2 + pr
                            head = (hch * MT + hmt) * 2 + hpr
                            if cur - head < lag:
                                break
                            pop_av()
            while queue:
                pop_av()

    nc.compile()
    return nc


_NC = None


def _get_nc():
    global _NC
    if _NC is None:
        _NC = build()
    return _NC


def _in_maps(x, w_theta, b_theta, w_phi, b_phi, w_g, b_g, w_z, b_z,
             bn_gamma, bn_beta, bn_mean, bn_var):
    sqa = np.float32(np.sqrt(ALPHA))
    inv = (np.asarray(bn_gamma, np.float32)
           / np.sqrt(np.asarray(bn_var, np.float32) + EPS))
    shift = ((np.asarray(w_z, np.float32) @ np.asarray(b_g, np.float32)
              + np.asarray(b_z, np.float32)) * inv
             + np.asarray(bn_beta, np.float32)
             - np.asarray(bn_mean, np.float32) * inv)

    def pack_w(w):  # [INTER, C] -> [128, 256] lhsT planes, fp8
        wT = np.asarray(w, np.float32).T           # [C, INTER]
        return np.concatenate([wT[:128], wT[128:]], axis=1)

    wq = pack_w(np.asarray(w_theta, np.float32) * sqa)
    wk = pack_w(np.asarray(w_phi, np.float32) * sqa)
    wg = pack_w(np.asarray(w_g, np.float32))
    wp8 = np.ascontiguousarray(
        np.concatenate([wq, wk, wg], axis=1)).astype(F8)
    wzs = np.ascontiguousarray(
        (np.asarray(w_z, np.float32) * inv[:, None]).T.astype(BF))  # [128,256]
    bqk = np.ascontiguousarray(np.stack(
        [np.asarray(b_theta, np.float32) * sqa,
         np.asarray(b_phi, np.float32) * sqa], axis=1))
    idn = np.eye(128, dtype=np.float32).astype(BF)

    xr = np.asarray(x, np.float32).reshape(B, C, N)
    shared = {"wp8": wp8, "wz": wzs, "bqk": bqk, "idn": idn}
    maps = []
    for core in range(NCORE):
        b_, half = divmod(core, 2)
        n0 = half * NH
        n1 = NH - n0
        xre = np.concatenate([xr[b_][:, n0:n0 + NH], xr[b_][:, n1:n1 + NH]],
                             axis=1)            # own query half first
        xs = xre.astype(F8)
        x8 = np.empty((128, 2 * N), F8)
        x8[:, 0:N] = xs[:128]
        x8[:, N:2 * N] = xs[128:]
        xqr = xre[:, 0:NH] + shift[:, None]
        xqp = np.concatenate([xqr[:128], xqr[128:]], axis=1).astype(np.float32)
        m = dict(shared)
        m["x8"] = np.ascontiguousarray(x8)
        m["xq"] = np.ascontiguousarray(xqp)
        maps.append(m)
    return maps


def kernel(**inputs):
    nc = _get_nc()
    maps = _in_maps(**inputs)
    res = bass_utils.run_bass_kernel_spmd(nc, maps, core_ids=list(range(NCORE)))
    out = np.empty((B, C, N), np.float32)
    for core in range(NCORE):
        b_, half = divmod(core, 2)
        n0 = half * NH
        out[b_][:, n0:n0 + NH] = res.results[core]["out"].reshape(C, NH)
    return out.reshape(B, C, H, W)
